# revision 75
# baseline (speedup 1.0000x reference)
"""Causal self-attention (B=4, T=2048, E=1024, H=16) on 8 trn2 NeuronCores.

Sharding: core c -> (batch b = c // 2, head-group hg = c % 2); each core owns
one batch element and 8 of the 16 heads (data parallel on B, tensor parallel
on heads).  No cross-core communication.

Engine-balanced v3 (ScalarE exp is the binding resource; everything else is
arranged to keep it fed):

  - All projection matmuls are fp8e4m3 MatmulPerfMode.DoubleRow (K = 2x128
    per instruction at 0.5 cycles/row), 3-term compensated:
    compensated with host-prepared fp8 residuals (2-term for q/k:
    x8@w8 + x8@dw8; 3-term for v, whose error feeds y directly).  Weights
    are pre-scaled by 16 so the dw residual clears the fp8 subnormal
    floor; biases are host-scaled by
    16 and the scale cancels via the softmax denominator (the "ones" column
    of v holds 16.0) and an exp scale of 0.125/256 (q and k both carry 16x).
  - q/k stored bf16; bias-add + PSUM->SBUF copy on DVE (stride-0 broadcast
    AP), keeping ScalarE exp-only.
  - exp trimmed on diagonal j-tiles to i >= 128r; one instruction per
    (pair, j-tile) covers both heads via a 3-D AP.  The gpsimd affine_select
    mask zero-fills the trimmed columns and applies the causal mask, both
    heads in one instruction.
  - PV for i-blocks I>=1: fp8 DoubleRow over j-tile pairs (K=256).  The
    stationary v8 tile is [128, NJ, 96]: cols 0-63 = 16v, col 64 = 16.0
    (denominator -> psum row 64), cols 65-95 zero (DoubleRow wants stationary
    free dim 64/96/128, k-tile pairs contiguous).  p-rounding errors cancel
    through p/sum(p); fp8 v is safe once softmax is diffuse (rows >= 512).
  - I=0 keeps PV in bf16 (concentrated-softmax rows) via a bf16 v copy of
    t-block 0.
  - Projection groups are woven into the attention stream by deadline
    (earliest attention item that reads them), smoothing PE load and letting
    attention-0 start ~25us earlier.
"""

import sys

sys.path.insert(0, "/opt/trn_rl_repo")

import numpy as np

N_CORES = 8
B, T, E = 4, 2048, 1024
H, D = 16, 64
C = E                 # q/k/v channel count (4th qkv chunk unused)
HPC = H // 2          # heads per core
CC = HPC * D          # per-core channels = 512
ES = E // 128         # 8 e-tiles (contraction)
EP = ES // 2          # 4 e-tile pairs (DoubleRow K=256)
TB = T // 512         # 4 t/i blocks of 512
NJ = T // 128         # 16 j-tiles of 128
PAIRS = HPC // 2      # 4 head pairs per core
VW = 96               # DoubleRow stationary width: 64 v + 1 ones + 31 pad
WSCALE = 16.0         # host weight/bias pre-scale
CTORDER = [0, 4, 1, 5, 2, 6, 3, 7]   # ct slots in wqk8, pair-major
CTPOS = {ct: s for s, ct in enumerate(CTORDER)}

_cache = {}


def _build_nc():
    import concourse.bass as bass
    import concourse.mybir as mybir
    import concourse.tile as tile
    from concourse import bacc

    f32 = mybir.dt.float32
    f32r = mybir.dt.float32r
    bf16 = mybir.dt.bfloat16
    f8 = mybir.dt.float8e4
    Act = mybir.ActivationFunctionType
    is_ge = mybir.AluOpType.is_ge
    DRmode = mybir.MatmulPerfMode.DoubleRow

    nc = bacc.Bacc("TRN2", target_bir_lowering=False, debug=False)

    # host-prepared fp8 operands (see shard_inputs for exact layouts)
    x8m_d = nc.dram_tensor("x8m", [128, EP, 2, T], f8, kind="ExternalInput").ap()
    dx8m_d = nc.dram_tensor("dx8m", [128, EP, 2, T], f8,
                            kind="ExternalInput").ap()
    x8s_d = nc.dram_tensor("x8s", [128, TB, EP, 4, 2, 128], f8,
                           kind="ExternalInput").ap()
    dx8s_d = nc.dram_tensor("dx8s", [128, TB, EP, 4, 2, 128], f8,
                            kind="ExternalInput").ap()
    wqk8_d = nc.dram_tensor("wqk8", [128, EP, 8, 2, 128], f8,
                            kind="ExternalInput").ap()
    dwqk8_d = nc.dram_tensor("dwqk8", [128, EP, 8, 2, 128], f8,
                             kind="ExternalInput").ap()
    wv8_d = nc.dram_tensor("wv8", [128, EP, 2, CC], f8, kind="ExternalInput").ap()
    dwv8_d = nc.dram_tensor("dwv8", [128, EP, 2, CC], f8,
                            kind="ExternalInput").ap()
    b_qk = nc.dram_tensor("b_qk", [128, 8], f32, kind="ExternalInput").ap()
    b_v = nc.dram_tensor("b_v", [1, CC], f32r, kind="ExternalInput").ap()
    ones_d = nc.dram_tensor("ones_d", [1, 128], f32r, kind="ExternalInput").ap()
    yT = nc.dram_tensor("yT", [CC, T], f32, kind="ExternalOutput").ap()

    def bcast_free(ap, n):
        """[P, 1] SBUF AP -> [P, n] via stride-0 free dim."""
        return bass.AP(
            tensor=ap.tensor, offset=ap.offset, ap=list(ap.ap)[:-1] + [[0, n]]
        )

    with tile.TileContext(nc) as tc:
        with (
            tc.tile_pool(name="persist", bufs=1) as pp,
            tc.tile_pool(name="psum", bufs=1, space="PSUM") as psp,
            tc.tile_pool(name="xpool", bufs=2) as xp,
            tc.tile_pool(name="ptpool", bufs=3) as ptp,
            tc.tile_pool(name="opool", bufs=1) as op,
            tc.tile_pool(name="dpool", bufs=2, space="DRAM") as dp,
        ):
            # ---- persistent SBUF state ----
            qk_sb = [pp.tile([128, T], bf16, name=f"qk{ct}") for ct in range(8)]
            v8_sb = pp.tile([128, HPC, NJ, VW], f8, name="v8")
            v0_sb = pp.tile([128, HPC, 4, D + 1], bf16, name="v0")
            bqk_sb = pp.tile([128, 8], f32, name="bqk")
            bv_sb = pp.tile([1, CC], f32r, name="bv")
            ones_sb = pp.tile([1, 128], f32r, name="ones")
            # weights: [e-part, e-pair, ct, ktile, 128ch] (qk) /
            #          [e-part, e-pair, ktile, 512ch] (v)
            wqk8 = pp.tile([128, EP, 8, 2, 128], f8, name="wqk8")
            dwqk8 = pp.tile([128, EP, 8, 2, 128], f8, name="dwqk8")
            wv8 = pp.tile([128, EP, 2, CC], f8, name="wv8")
            dwv8 = pp.tile([128, EP, 2, CC], f8, name="dwv8")

            # PE warm-up first (memset on the otherwise-idle DVE so it runs
            # at t~0): ~5us of junk matmuls burn the p-state ramp before the
            # first projection group, which otherwise runs at 0.65-1.2GHz
            warm_sb = pp.tile([1, 512], bf16, name="warm")
            nc.vector.memset(warm_sb, 0.0)
            for wi in range(7):
                wps = psp.tile([128, 512], f32, tag="st", bufs=2,
                               name=f"warm{wi}")
                nc.tensor.matmul(wps, warm_sb[:, 0:128], warm_sb,
                                 start=True, stop=True)

            # ones (16x) and zero pad of the stationary v tiles
            nc.gpsimd.memset(v8_sb[:, :, :, D : D + 1], WSCALE)
            nc.gpsimd.memset(v8_sb[:, :, :, D + 1 : VW], 0.0)
            nc.gpsimd.memset(v0_sb[:, :, :, D : D + 1], WSCALE)
            # exp bias constant (keeps unnormalized p under fp8e4m3 max)
            expb_sb = pp.tile([128, 1], f32, name="expb")
            nc.gpsimd.memset(expb_sb, -3.0)
            # dummy exp so the Act table load happens at t~0, not in the
            # middle of the first attention item
            junk8 = pp.tile([1, 16], f8, name="junk8")
            nc.scalar.activation(junk8, warm_sb[0:1, 0:16], Act.Exp)
            # bf16 ones for the PE-broadcast in the final pair's out_stage
            ones32_sb = pp.tile([65, 64], bf16, name="ones32")
            nc.vector.memset(ones32_sb, 1.0)

            # (b_qk/b_v/ones DMAs issued below, after the first x/w tiles
            # that gate the first exp)

            # x tiles per t-block: moving layout [128, ep, ktile, 512t] and
            # stationary layout [128, ep, k4, ktile, 128t]
            xs_tb = {}

            def load_x(tb, stationary_too=True):
                tsl = slice(tb * 512, (tb + 1) * 512)
                xm = xp.tile([128, EP, 2, 512], f8, tag="xm", bufs=2,
                             name=f"xm{tb}")
                dxm = xp.tile([128, EP, 2, 512], f8, tag="dxm", bufs=2,
                              name=f"dxm{tb}")
                xs = xp.tile([128, EP, 4, 2, 128], f8, tag="xs", bufs=2,
                             name=f"xs{tb}")
                dxs = xp.tile([128, EP, 4, 2, 128], f8, tag="dxs", bufs=2,
                              name=f"dxs{tb}")
                nc.sync.dma_start(out=xm, in_=x8m_d[:, :, :, tsl])
                nc.sync.dma_start(out=dxm, in_=dx8m_d[:, :, :, tsl])
                if stationary_too:
                    nc.sync.dma_start(out=xs, in_=x8s_d[:, tb])
                    nc.sync.dma_start(out=dxs, in_=dx8s_d[:, tb])
                xs_tb[tb] = (xm, dxm, xs, dxs)

            # tb0: q/k operands first (they gate the first exp): the ct dim
            # is host-ordered [0,4,1,5,2,6,3,7] so the first DMA half covers
            # head-pairs 0-1
            load_x(0, stationary_too=False)
            nc.sync.dma_start(out=wqk8[:, :, 0:4], in_=wqk8_d[:, :, 0:4])
            nc.sync.dma_start(out=dwqk8[:, :, 0:4], in_=dwqk8_d[:, :, 0:4])
            nc.sync.dma_start(out=bqk_sb, in_=b_qk)
            _, _, xs0, dxs0 = xs_tb[0]
            nc.sync.dma_start(out=xs0, in_=x8s_d[:, 0])
            nc.sync.dma_start(out=dxs0, in_=dx8s_d[:, 0])
            nc.sync.dma_start(out=wv8, in_=wv8_d)
            nc.sync.dma_start(out=dwv8, in_=dwv8_d)
            nc.sync.dma_start(out=bv_sb, in_=b_v)
            nc.sync.dma_start(out=ones_sb, in_=ones_d)
            nc.sync.dma_start(out=wqk8[:, :, 4:8], in_=wqk8_d[:, :, 4:8])
            nc.sync.dma_start(out=dwqk8[:, :, 4:8], in_=dwqk8_d[:, :, 4:8])

            # ---- projection groups (contiguous emission per group) ----
            def qkv_qk(tb, ct):
                tsl = slice(tb * 512, (tb + 1) * 512)
                xm, dxm, _, _ = xs_tb[tb]
                ps = psp.tile([128, 512], f32, tag="st", bufs=2,
                              name=f"psqk{ct}_{tb}")
                cs = CTPOS[ct]
                terms = [(wqk8, xm), (wqk8, dxm), (dwqk8, xm)]
                n = 0
                for wt, xt in terms:
                    for ep in range(EP):
                        n += 1
                        nc.tensor.matmul(
                            ps, wt[:, ep, cs], xt[:, ep],
                            start=(n == 1), stop=(n == 3 * EP),
                            perf_mode=DRmode, skip_group_check=True,
                        )
                nc.vector.tensor_add(
                    qk_sb[ct][:, tsl], ps, bcast_free(bqk_sb[:, ct : ct + 1], 512)
                )

            def qkv_v(tb, k4):
                _, _, xs, dxs = xs_tb[tb]
                tt = tb * 4 + k4
                psv = psp.tile([128, 512], f32, tag="st", bufs=2,
                               name=f"psv{tt}")
                # 16*b_v via a K=1 matmul (ones column trick), then the
                # 3-term fp8 DoubleRow accumulation
                nc.tensor.matmul(
                    psv, ones_sb, bv_sb,
                    start=True, stop=False, skip_group_check=True,
                )
                terms = [(xs, wv8), (dxs, wv8), (xs, dwv8)]
                n = 0
                for xt, wt in terms:
                    for ep in range(EP):
                        n += 1
                        nc.tensor.matmul(
                            psv, xt[:, ep, k4], wt[:, ep],
                            start=False, stop=(n == 3 * EP),
                            perf_mode=DRmode, skip_group_check=True,
                        )
                pv8 = psv.rearrange("p (h d) -> p h d", d=D)
                nc.vector.tensor_copy(v8_sb[:, :, tt, 0:D], pv8)
                if tb == 0:
                    nc.vector.tensor_copy(v0_sb[:, :, tt, 0:D], pv8)

            # ---- flat cross-block schedule: a single item stream so block
            # boundaries pipeline (I+1's QK/exp work fills I's PE-heavy
            # projection windows) ----
            ibase = [0, 8, 24, 48]

            def item_index(I, pr, jp):
                return ibase[I] + pr * (2 * I + 2) + jp

            items_all = [(I, pr, jp) for I in range(TB)
                         for pr in range(PAIRS) for jp in range(2 * I + 2)]
            yts = {}
            pts = {}

            def qk_one(I, pr, J, pt):
                qt = qk_sb[pr]
                kt = qk_sb[4 + pr]
                jsl = slice(J * 128, (J + 1) * 128)
                r = J - 4 * I
                istart = 128 * r if r > 0 else 0    # diagonal trim
                w = 512 - istart
                isl = slice(I * 512 + istart, (I + 1) * 512)
                jm = J % 2
                st = psp.tile([128, 1024], f32, tag="st", bufs=2,
                              name=f"st{pr}_{I}_{J}")
                nc.tensor.matmul(
                    st[:, istart : istart + w], kt[0:64, jsl], qt[0:64, isl],
                    tile_position=(0, 0),
                )
                nc.tensor.matmul(
                    st[:, 512 + istart : 512 + istart + w],
                    kt[64:128, jsl], qt[64:128, isl],
                    tile_position=(64, 0),
                )
                st3 = st.rearrange("p (h i) -> p h i", i=512)
                # bias -3 keeps unnormalized p under the fp8e4m3 max of
                # 240 (cancels in the softmax ratio)
                nc.scalar.activation(
                    pt[:, :, jm, istart:512], st3[:, :, istart:512],
                    Act.Exp, bias=expb_sb, scale=0.125 / (WSCALE * WSCALE),
                )
                if r >= 0:
                    # true-diagonal 128-wide causal mask; the zero-fill of
                    # the exp-trimmed columns [0, 128r) was issued at pt
                    # alloc time (off the exp->PV critical path)
                    dsl = slice(128 * r, 128 * (r + 1))
                    nc.gpsimd.affine_select(
                        out=pt[:, :, jm, dsl],
                        in_=pt[:, :, jm, dsl],
                        compare_op=is_ge,
                        fill=0.0,
                        base=0,
                        pattern=[[0, 2], [1, 128]],
                        channel_multiplier=-1,
                    )

            def emit_item(g):
                I, pr, jp = items_all[g]
                fp8 = I > 0
                if jp == 0:
                    yts[(I, pr)] = [
                        psp.tile([128, 512], f32, tag=f"yt{h}", bufs=2,
                                 name=f"yt{h}_{pr}_{I}")
                        for h in range(2)
                    ]
                pt = ptp.tile(
                    [128, 2, 2, 512], f8 if fp8 else bf16,
                    tag=("pt" if fp8 else "pt0"), bufs=6,
                    name=f"pt{pr}_{I}_{jp}",
                )
                pts[(I, pr, jp)] = pt
                # zero-fill exp-trimmed columns of diagonal tiles now,
                # while the Pool engine is idle anyway
                for jj in range(2):
                    r = 2 * jp + jj - 4 * I
                    if r > 0:
                        nc.gpsimd.memset(pt[:, :, jj, 0 : 128 * r], 0.0)
                qk_one(I, pr, 2 * jp, pt)
                qk_one(I, pr, 2 * jp + 1, pt)

            def pv(I, pr, jp):
                pt = pts.pop((I, pr, jp))
                nj = 4 * I + 4
                first, last = (jp == 0), (jp == nj // 2 - 1)
                for h in range(2):
                    hh = 2 * pr + h
                    yt = yts[(I, pr)][h]
                    if I > 0:
                        nc.tensor.matmul(
                            yt[0:VW, :],
                            v8_sb[:, hh, 2 * jp : 2 * jp + 2, :],
                            pt[:, h, :, :],
                            start=first, stop=last,
                            perf_mode=DRmode, skip_group_check=True,
                        )
                    else:
                        for jj in range(2):
                            J = 2 * jp + jj
                            nc.tensor.matmul(
                                yt[0 : D + 1, :],
                                v0_sb[:, hh, J, :],
                                pt[:, h, jj, :],
                                start=(J == 0), stop=(J == nj - 1),
                                skip_group_check=True,
                            )

            def out_stage_a(I, pr):
                """Reciprocals; broadcast via DRAM bounce except in the last
                block, where a K=1 matmul broadcast (the PE idles there)
                shortens the drain chain."""
                ytA, ytB = yts.pop((I, pr))
                rec2 = op.tile([65, 512], f32, tag="rec2", bufs=2,
                               name=f"rec2{pr}_{I}")
                recA, recB = rec2[0:1, :], rec2[64:65, :]
                nc.vector.reciprocal(recA, ytA[64:65, :])
                nc.vector.reciprocal(recB, ytB[64:65, :])
                if I == TB - 1 and pr == PAIRS - 1:
                    # final pair: avoid the DRAM bounce latency entirely --
                    # bf16 reciprocal copies + K=1 PE broadcast + SBUF y
                    # copies (so the muls read only one PSUM operand)
                    recb = op.tile([65, 512], bf16, tag="recb",
                                   name=f"recb{pr}_{I}")
                    nc.vector.tensor_copy(recb[0:1, :], recA)
                    nc.vector.tensor_copy(recb[64:65, :], recB)
                    y_sb = op.tile([128, 512], f32, tag="ysb",
                                   name=f"ysb{pr}_{I}")
                    nc.vector.tensor_copy(y_sb[0:64, :], ytA[0:D, :])
                    nc.vector.tensor_copy(y_sb[64:128, :], ytB[0:D, :])
                    rbp = psp.tile([64, 1024], f32, tag="st", bufs=2,
                                   name=f"rbp{pr}_{I}")
                    nc.tensor.matmul(rbp[:, 0:512], ones32_sb[0:1, :],
                                     recb[0:1, :], start=True, stop=True,
                                     skip_group_check=True)
                    nc.tensor.matmul(rbp[:, 512:1024], ones32_sb[64:65, :],
                                     recb[64:65, :], start=True, stop=True,
                                     skip_group_check=True)
                    return (y_sb, rbp, None)
                recA_d = dp.tile([1, 512], f32, tag="recA_d",
                                 name=f"recAd{pr}_{I}")
                recB_d = dp.tile([1, 512], f32, tag="recB_d",
                                 name=f"recBd{pr}_{I}")
                rbc2 = op.tile([128, 512], f32, tag="rbc2", bufs=2,
                               name=f"rbc2{pr}_{I}")
                nc.sync.dma_start(out=recA_d, in_=recA)
                nc.sync.dma_start(out=recB_d, in_=recB)
                nc.sync.dma_start(out=rbc2[0:64, :], in_=_bcast_ap(recA_d, 64))
                nc.sync.dma_start(out=rbc2[64:128, :], in_=_bcast_ap(recB_d, 64))
                return (ytA, ytB, rbc2)

            def out_stage_b(I, pr, st8):
                """Deferred normalize+store: by now the broadcast has landed,
                so the DVE muls don't head-of-line block the DVE queue (which
                would stall projection bias-adds and thus exp)."""
                ytA, ytB, rbc2 = st8
                isl = slice(I * 512, (I + 1) * 512)
                ystage = op.tile([128, 512], f32, tag="ystage", bufs=2,
                                 name=f"ys{pr}_{I}")
                if rbc2 is None:
                    y_sb, rbp = ytA, ytB
                    nc.vector.tensor_mul(ystage[0:64, :], y_sb[0:64, :],
                                         rbp[:, 0:512])
                    nc.vector.tensor_mul(ystage[64:128, :], y_sb[64:128, :],
                                         rbp[:, 512:1024])
                else:
                    nc.vector.tensor_mul(ystage[0:64, :], ytA[0:D, :],
                                         rbc2[0:64, :])
                    nc.vector.tensor_mul(ystage[64:128, :], ytB[0:D, :],
                                         rbc2[64:128, :])
                nc.sync.dma_start(
                    out=yT[pr * 128 : (pr + 1) * 128, isl], in_=ystage)

            def tb_groups(tb):
                # q/k groups forced before their first QK reader (margin so
                # the DVE bias-add clears first); v groups forced before
                # their first PV reader (which trails emission by the
                # lookahead, so a small margin suffices)
                mqk = 1 if tb == 0 else 5
                qks, vs = [], []
                for ct in range(8):
                    # stagger k-side groups 2 items earlier than q-side so a
                    # pair's two groups don't burst back-to-back
                    qks.append(
                        (item_index(tb, ct % 4, 0) - mqk - (2 if ct >= 4 else 0),
                         (lambda t=tb, c=ct: qkv_qk(t, c)))
                    )
                for k4 in range(4):
                    # v deadlines are PV-item indices, staggered within the
                    # pair so two groups never burst back-to-back; forced
                    # from the emission loop ahead of their PV
                    vs.append(
                        (item_index(tb, 0, 2 * tb + k4 // 2) - (k4 % 2),
                         (lambda t=tb, k=k4: qkv_v(t, k)))
                    )
                qks.sort(key=lambda g: g[0])
                vs.sort(key=lambda g: g[0])
                return qks, vs

            state = {"woven": 0, "gk": 0}
            queue, vqueue = tb_groups(0)

            def force_due(q, limit):
                while q and q[0][0] <= limit:
                    q.pop(0)[1]()
                    state["woven"] += 1

            G = len(items_all)
            LOOKAHEAD = 5
            emitted = 0
            pending_b = []
            for k in range(G):
                while emitted < min(k + LOOKAHEAD, G):
                    gI, gpr, gjp = items_all[emitted]
                    if gpr == 0 and gjp == 0 and gI + 1 < TB:
                        # emission crosses into block gI: stage tb gI+1
                        load_x(gI + 1)
                        q2, v2 = tb_groups(gI + 1)
                        queue.extend(q2)
                        vqueue.extend(v2)
                        queue.sort(key=lambda g: g[0])
                        vqueue.sort(key=lambda g: g[0])
                    force_due(queue, emitted)
                    emit_item(emitted)
                    emitted += 1
                I, pr, jp = items_all[k]
                force_due(vqueue, k)
                pv(I, pr, jp)
                while pending_b and pending_b[0][0] <= k:
                    _, bI, bpr, st8 = pending_b.pop(0)
                    out_stage_b(bI, bpr, st8)
                if jp == 2 * I + 1:
                    pending_b.append((k + 2, I, pr, out_stage_a(I, pr)))
                # proactive smoothing: ~0.6 groups per attention item
                state["gk"] += 1
                while queue and state["woven"] < (state["gk"] * 6) // 10:
                    queue.pop(0)[1]()
                    state["woven"] += 1
            for _, bI, bpr, st8 in pending_b:
                out_stage_b(bI, bpr, st8)
            for _, fn in queue + vqueue:
                fn()
    nc.compile()
    return nc


def _bcast_ap(src_ap, nparts):
    """Partition-broadcast view of a [1, N] DRAM AP -> [nparts, N]."""
    import concourse.bass as bass

    return bass.AP(
        tensor=src_ap.tensor,
        offset=src_ap.offset,
        ap=[[0, nparts]] + list(src_ap.ap)[1:],
    )


def get_nc():
    if "nc" not in _cache:
        _cache["nc"] = _build_nc()
    return _cache["nc"]


def _to_f8(a):
    import ml_dtypes

    return a.astype(ml_dtypes.float8_e4m3)


def _f8_pair(a):
    """fp8 value + fp8 residual of a float32 array."""
    a8 = _to_f8(a)
    da8 = _to_f8(a - a8.astype(np.float32))
    return a8, da8


def shard_inputs(x, w_attn, b_attn):
    """Full inputs -> per-core input maps (host-side slicing, transposition,
    fp8 rounding + residuals, weight 16x pre-scale, b_v folded into an
    appended x ones-row)."""
    x = np.asarray(x, dtype=np.float32)
    w = np.asarray(w_attn, dtype=np.float32)
    bb = np.asarray(b_attn, dtype=np.float32)
    in_maps = []
    for core in range(N_CORES):
        b, hg = core // 2, core % 2
        r0 = hg * CC

        # x in [E, T] (channels on partitions)
        xT = np.ascontiguousarray(x[b].T)                       # [E, T]
        x8, dx8 = _f8_pair(xT)
        # moving layout [128, EP, 2, T]: (p, ep, i, t) = x[(2ep+i)*128+p, t]
        x8m = np.ascontiguousarray(
            x8.reshape(EP, 2, 128, T).transpose(2, 0, 1, 3))
        dx8m = np.ascontiguousarray(
            dx8.reshape(EP, 2, 128, T).transpose(2, 0, 1, 3))
        # stationary layout [128, TB, EP, 4, 2, 128]:
        # (p, tb, ep, k4, i, m) = x[(2ep+i)*128+p, tb*512+k4*128+m]
        xs6 = x8.reshape(EP, 2, 128, TB, 4, 128).transpose(2, 3, 0, 4, 1, 5)
        dxs6 = dx8.reshape(EP, 2, 128, TB, 4, 128).transpose(2, 3, 0, 4, 1, 5)

        # q/k weights [E, 2*CC] scaled by 16, transposed to [E(contract), ch]
        wqkT = np.concatenate(
            [w[r0 : r0 + CC, :], w[C + r0 : C + r0 + CC, :]], axis=0
        ).T * WSCALE                                            # [E, 1024]
        w8, dw8 = _f8_pair(wqkT)
        # [128, EP, 8, 2, 128]: (p, ep, s, i, m) = w[(2ep+i)*128+p,
        # CTORDER[s]*128+m] -- ct slots pair-major so the first DMA half
        # covers head-pairs 0-1
        wq5 = w8.reshape(EP, 2, 128, 8, 128).transpose(2, 0, 3, 1, 4)[:, :, CTORDER]
        dwq5 = dw8.reshape(EP, 2, 128, 8, 128).transpose(2, 0, 3, 1, 4)[:, :, CTORDER]

        # v weights [E, CC] scaled by 16 -> [128, EP, 2, CC]
        wvT = w[2 * C + r0 : 2 * C + r0 + CC, :].T * WSCALE     # [E, 512]
        wv8, dwv8 = _f8_pair(wvT)
        wv4 = wv8.reshape(EP, 2, 128, CC).transpose(2, 0, 1, 3)
        dwv4 = dwv8.reshape(EP, 2, 128, CC).transpose(2, 0, 1, 3)

        b_qk = np.stack(
            [bb[r0 + ct * 128 : r0 + (ct + 1) * 128] for ct in range(4)]
            + [bb[C + r0 + ct * 128 : C + r0 + (ct + 1) * 128] for ct in range(4)],
            axis=1,
        ).astype(np.float32) * WSCALE
        b_v = (bb[2 * C + r0 : 2 * C + r0 + CC] * WSCALE).reshape(1, CC)

        in_maps.append(
            {
                "x8m": x8m,
                "dx8m": dx8m,
                "x8s": np.ascontiguousarray(xs6),
                "dx8s": np.ascontiguousarray(dxs6),
                "wqk8": np.ascontiguousarray(wq5),
                "dwqk8": np.ascontiguousarray(dwq5),
                "wv8": np.ascontiguousarray(wv4),
                "dwv8": np.ascontiguousarray(dwv4),
                "b_qk": np.ascontiguousarray(b_qk),
                "b_v": np.ascontiguousarray(b_v.astype(np.float32)),
                "ones_d": np.ones((1, 128), dtype=np.float32),
            }
        )
    return in_maps


def run(in_maps, trace=False, **kw):
    from concourse import bass_utils

    nc = get_nc()
    return bass_utils.run_bass_kernel_spmd(
        nc, in_maps, core_ids=list(range(N_CORES)), trace=trace, **kw
    )


def gather_output(results):
    y = np.empty((B, T, E), dtype=np.float32)
    for core in range(N_CORES):
        b, hg = core // 2, core % 2
        y[b, :, hg * CC : (hg + 1) * CC] = results[core]["yT"].T
    return y


def kernel(x, w_attn, b_attn):
    in_maps = shard_inputs(x, w_attn, b_attn)
    res = run(in_maps, trace=False)
    return gather_output(res.results)
